# revision 30
# baseline (speedup 1.0000x reference)
"""Exphormer attention (GNN message passing) Trainium2 Bass kernel, v3.

Strategy (dst-sharded, zero collectives):
  - Core m owns nodes [m*12500, (m+1)*12500) and all edges pointing into
    them; each core computes its output slice independently.
  - All model compute (K/V/Q/Ef projections, scores, exp, messages,
    scatter-add, normalization) runs on device.  The host prepares index
    bookkeeping only: edge ordering, per-edge operand staging (edge_attr
    rows and x[src] rows laid out subtile-major in bf16), one-hot routing
    matrices (fp8 0/1 encodings of the dst indices), and bf16/transposed
    copies of the weights.
  - Edges are grouped by dst-chunk (128 nodes) into 128-edge subtiles
    (padded, subtile count uniform across cores).  Per subtile one PE
    matmul computes [Ef|K|V] = [ea|x_src]^T @ blockdiag(WE, WK|WV); a
    second (fp8 one-hot lhsT) computes Qd = M_T.T @ Qchunk.
  - score = exp(clip(sum_dh K*Ef*Qd)): products on DVE, per-head reduce +
    clip on GPSIMD, exp (broadcast over DH) on ACT.
  - Scatter: one matmul per subtile with lhsT = one-hot M (fp8) and
    rhs = [V*score | score] accumulates wV|Z node-major in PSUM; chunk
    epilogue divides by Z+eps and stores node-major.
"""

import sys

import numpy as np

sys.path.insert(0, "/opt/trn_rl_repo")

import ml_dtypes  # noqa: E402

BF16 = ml_dtypes.bfloat16
FP8 = ml_dtypes.float8_e4m3
FP8_ONE = np.uint8(0x38)  # 1.0 in e4m3

# ---------------- problem geometry (hardcoded per contract) ----------------
N = 100000
NE = 1250000
D = 64
H = 8
DH = 8
NCORES = 8
NPC = N // NCORES          # 12500 nodes per core
CHUNK = 128                # nodes per dst-chunk
NCHUNK = (NPC + CHUNK - 1) // CHUNK   # 98
NPAD = NCHUNK * CHUNK      # 12544
SUB = 128                  # edges per subtile
BATCH = 8                  # subtiles per compute batch
EXP_CLIP = 5.0


# ---------------- host-side preprocessing ----------------
def _preprocess(x, edge_attr, WQ, WK, WV, WE, edge_index):
    src = np.ascontiguousarray(edge_index[0]).astype(np.int64)
    dst = np.ascontiguousarray(edge_index[1]).astype(np.int64)
    core_of = dst // NPC
    dloc_all = dst - core_of * NPC
    chunk_all = dloc_all // CHUNK

    order = np.lexsort((src, chunk_all, core_of))
    key_s = (core_of * NCHUNK + chunk_all)[order]

    cnt = np.bincount(key_s, minlength=NCORES * NCHUNK).reshape(
        NCORES, NCHUNK)
    # subtiles per chunk: uniform across cores, rounded up to EVEN counts
    S = np.ceil(cnt.max(axis=0) / SUB).astype(np.int64)
    S = S + (S % 2)

    cell_st = np.concatenate([[0], np.cumsum(S)]).astype(np.int64)
    ts = int(cell_st[-1])

    # compute batches ("calls"): <=BATCH-subtile windows within each chunk
    calls = []            # (c, st, ns)
    for c in range(NCHUNK):
        o = 0
        while o < S[c]:
            ns = min(BATCH, int(S[c]) - o)
            calls.append((c, int(cell_st[c]) + o, ns))
            o += ns

    sub_chunk = np.zeros(ts, dtype=np.int64)
    for c in range(NCHUNK):
        sub_chunk[cell_st[c]:cell_st[c + 1]] = c
    chunk_first = cell_st[:-1].copy()
    chunk_last = cell_st[1:] - 1

    geom = dict(ts=ts, calls=calls, sub_chunk=sub_chunk,
                chunk_first=chunk_first, chunk_last=chunk_last)

    # ---- per-core data staging ----
    E_pad = ts * SUB
    src_s = src[order]
    dloc_s = dloc_all[order]
    chunk_s = chunk_all[order]
    core_s = core_of[order]
    core_starts = np.searchsorted(core_s, np.arange(NCORES + 1))

    x_bf = x.astype(BF16)
    per_core = []
    for m in range(NCORES):
        lo, hi = core_starts[m], core_starts[m + 1]
        c_src = src_s[lo:hi]
        c_dloc = dloc_s[lo:hi]
        c_chunk = chunk_s[lo:hi]
        c_eid = order[lo:hi]

        run_starts = np.searchsorted(c_chunk, np.arange(NCHUNK + 1))
        pos = np.arange(hi - lo) - run_starts[c_chunk]
        gslot = cell_st[c_chunk] * SUB + pos           # global edge slot

        # combined input slab [128, ts*512] (bytes): per subtile g the
        # 512-byte block holds [ea|x_src] bf16 (256B: rows 0:64 =
        # edge_attr[e].T, rows 64:128 = x[src(e)].T), then the one-hot
        # fp8 M_T (128B: mt[n,e] = dloc_local(e)==n) and M (128B:
        # m[e,n] = dloc_local(e)==n).
        dll = (c_dloc - c_chunk * CHUNK).astype(np.int64)   # 0..127
        gs, ge = gslot // SUB, gslot % SUB
        comb = np.zeros((128, ts * 512), dtype=np.uint8)
        cb16 = comb.view('<u2')                             # [128, ts*256]
        ea16 = np.ascontiguousarray(
            edge_attr[c_eid].T.astype(BF16)).view('<u2')
        xg16 = np.ascontiguousarray(x_bf[c_src].T).view('<u2')
        cb16[0:D, gs * 256 + ge] = ea16
        cb16[D:128, gs * 256 + ge] = xg16
        comb[dll, gs * 512 + 256 + ge] = FP8_ONE
        comb[ge, gs * 512 + 384 + dll] = FP8_ONE

        n0 = m * NPC
        xq = np.zeros((NPAD, D), dtype=np.float32)
        xq[:NPC] = x[n0:n0 + NPC]
        xtq = np.ascontiguousarray(xq.T).astype(BF16)

        per_core.append(dict(comb=comb.view(FP8), xtq=xtq))

    # block-diagonal projection weights [128, 192]:
    #   rows 0:64  -> [WE | 0 | 0], rows 64:128 -> [0 | WK | WV]
    wkve = np.zeros((128, 3 * D), dtype=BF16)
    wkve[0:D, 0:D] = WE.astype(BF16)
    wkve[D:128, D:2 * D] = WK.astype(BF16)
    wkve[D:128, 2 * D:3 * D] = WV.astype(BF16)
    wq = (WQ / np.sqrt(DH)).astype(BF16)

    shared = dict(wkve=wkve, wq=wq)
    return per_core, shared, geom


# ---------------- device program ----------------
def _build_program(geom):
    from contextlib import ExitStack

    from concourse import bacc, mybir
    import concourse.tile as tile

    ts = geom["ts"]
    calls = geom["calls"]
    chunk_first = geom["chunk_first"]
    chunk_last = geom["chunk_last"]

    dt = mybir.dt
    nc = bacc.Bacc("TRN2", target_bir_lowering=False, debug=False,
                   num_devices=NCORES)

    xtq = nc.dram_tensor("xtq", [D, NPAD], dt.bfloat16,
                         kind="ExternalInput").ap()
    wkve_d = nc.dram_tensor("wkve", [128, 3 * D], dt.bfloat16,
                            kind="ExternalInput").ap()
    wq_d = nc.dram_tensor("wq", [D, D], dt.bfloat16, kind="ExternalInput").ap()
    comb_d = nc.dram_tensor("comb", [128, ts * 512], dt.float8e4,
                            kind="ExternalInput").ap()
    out_d = nc.dram_tensor("out", [NPAD, D], dt.float32,
                           kind="ExternalOutput").ap()

    with tile.TileContext(nc) as tc, ExitStack() as ctx:
        const_p = ctx.enter_context(tc.tile_pool(name="const", bufs=1))
        sb_pre = ctx.enter_context(tc.tile_pool(name="sb_pre", bufs=3))
        eax_p = ctx.enter_context(tc.tile_pool(name="eax", bufs=4))
        mt_p = ctx.enter_context(tc.tile_pool(name="mt", bufs=4))
        sb = ctx.enter_context(tc.tile_pool(name="sb", bufs=3))
        ep_p = ctx.enter_context(tc.tile_pool(name="ep", bufs=3))
        psK = ctx.enter_context(tc.tile_pool(name="psK", bufs=2, space="PSUM"))
        psB = ctx.enter_context(tc.tile_pool(name="psB", bufs=2, space="PSUM"))
        ps_acc = ctx.enter_context(
            tc.tile_pool(name="ps_acc", bufs=2, space="PSUM"))

        wkve_t = const_p.tile([128, 3 * D], dt.bfloat16)
        nc.sync.dma_start(out=wkve_t[:], in_=wkve_d)
        wq_t = const_p.tile([D, D], dt.bfloat16)
        nc.sync.dma_start(out=wq_t[:], in_=wq_d)

        # ---- pre-pass: Q table resident in SBUF ----
        qtab = const_p.tile([128, NCHUNK, D], dt.bfloat16)
        for c0 in range(0, NCHUNK, 4):
            n4 = min(4, NCHUNK - c0)
            xq_t = sb_pre.tile([D, 4 * SUB], dt.bfloat16, tag="xq_t")
            nc.sync.dma_start(out=xq_t[:, 0:n4 * SUB],
                              in_=xtq[:, c0 * SUB:(c0 + n4) * SUB])
            q_ps = psB.tile([128, BATCH, D], dt.float32, tag="qd")
            for bi in range(n4):
                nc.tensor.matmul(out=q_ps[:, bi, :],
                                 lhsT=xq_t[:, bi * SUB:(bi + 1) * SUB],
                                 rhs=wq_t[:], start=True, stop=True)
            nc.vector.tensor_copy(out=qtab[:, c0:c0 + n4, :],
                                  in_=q_ps[:, 0:n4, :])

        # ---- main loop over compute batches ----
        acc = None
        for ci, (c, cst, ns) in enumerate(calls):
            comb_t = eax_p.tile([128, BATCH * 512], dt.float8e4, tag="comb")
            nc.sync.dma_start(out=comb_t[:, 0:ns * 512],
                              in_=comb_d[:, cst * 512:(cst + ns) * 512])

            def eax_j(j):
                return comb_t[:, j * 512:j * 512 + 256].bitcast(dt.bfloat16)

            def mt_j(j):
                return comb_t[:, j * 512 + 256:j * 512 + 384]

            def m_j(j):
                return comb_t[:, j * 512 + 384:j * 512 + 512]

            if cst == chunk_first[c]:
                acc = ps_acc.tile([128, D + H], dt.float32,
                                  name=f"acc{c}", tag="acc")

            # [Ef|K|V|Qd] per subtile, in half-batches of 4; the 256-float
            # stride keeps each matmul output slice within one 2KB bank
            ekv_sb = sb.tile([128, BATCH, 256], dt.bfloat16, tag="ekv")
            for b0 in range(0, ns, 4):
                hs = min(4, ns - b0)
                ekv_ps = psK.tile([128, 4, 256], dt.float32, tag="ekv_ps")
                for j in range(hs):
                    nc.tensor.matmul(
                        out=ekv_ps[:, j, 0:3 * D],
                        lhsT=eax_j(b0 + j),
                        rhs=wkve_t[:], start=True, stop=True)
                    nc.tensor.matmul(
                        out=ekv_ps[:, j, 3 * D:4 * D],
                        lhsT=mt_j(b0 + j),
                        rhs=qtab[:, c, :], start=True, stop=True)
                nc.scalar.copy(out=ekv_sb[:, b0:b0 + hs, :],
                               in_=ekv_ps[:, 0:hs, :])

            t1_t = sb.tile([128, BATCH, D], dt.bfloat16, tag="t1")
            nc.vector.tensor_tensor(out=t1_t[:, 0:ns, :],
                                    in0=ekv_sb[:, 0:ns, D:2 * D],
                                    in1=ekv_sb[:, 0:ns, 0:D],
                                    op=mybir.AluOpType.mult)
            s2_t = sb.tile([128, BATCH, D], dt.bfloat16, tag="s2")
            nc.vector.tensor_tensor(out=s2_t[:, 0:ns, :],
                                    in0=t1_t[:, 0:ns, :],
                                    in1=ekv_sb[:, 0:ns, 3 * D:4 * D],
                                    op=mybir.AluOpType.mult)
            sc_t = sb.tile([128, BATCH, H], dt.float32, tag="sc")
            nc.vector.tensor_reduce(
                out=sc_t[:, 0:ns, :],
                in_=s2_t[:, 0:ns, :].rearrange("p m (h d) -> p m h d", d=DH),
                axis=mybir.AxisListType.X, op=mybir.AluOpType.add)
            scc_t = sb.tile([128, BATCH, H], dt.float32, tag="scc")
            nc.gpsimd.tensor_scalar(
                out=scc_t[:, 0:ns, :], in0=sc_t[:, 0:ns, :],
                scalar1=EXP_CLIP, scalar2=-EXP_CLIP,
                op0=mybir.AluOpType.min, op1=mybir.AluOpType.max)
            # exp with DH-broadcast on ACT: se_rep[e, m, h*8+d] = exp(scc)
            se_rep = sb.tile([128, BATCH, D], dt.bfloat16, tag="serep")
            nc.scalar.activation(
                out=se_rep[:, 0:ns, :].rearrange("p m (h d) -> p m h d",
                                                 d=DH),
                in_=scc_t[:, 0:ns, :].unsqueeze(3).to_broadcast(
                    [128, ns, H, DH]),
                func=mybir.ActivationFunctionType.Exp)
            # payload [V*score | score] so one matmul accumulates wV and Z
            pl_t = sb.tile([128, BATCH, D + H], dt.bfloat16, tag="pl")
            nc.gpsimd.tensor_tensor(out=pl_t[:, 0:ns, 0:D],
                                    in0=ekv_sb[:, 0:ns, 2 * D:3 * D],
                                    in1=se_rep[:, 0:ns, :],
                                    op=mybir.AluOpType.mult)
            nc.vector.tensor_copy(
                out=pl_t[:, 0:ns, D:D + H],
                in_=se_rep[:, 0:ns, :].rearrange("p m (h d) -> p m h d",
                                                 d=DH)[:, :, :, 0])

            for j in range(ns):
                g = cst + j
                nc.tensor.matmul(
                    out=acc[:],
                    lhsT=m_j(j),
                    rhs=pl_t[:, j, :],
                    start=bool(g == chunk_first[c]),
                    stop=bool(g == chunk_last[c]))

            if cst + ns - 1 == chunk_last[c]:
                ze_t = ep_p.tile([CHUNK, H], dt.float32, tag="ze")
                nc.scalar.activation(
                    out=ze_t[:], in_=acc[:, D:D + H],
                    func=mybir.ActivationFunctionType.Copy, bias=1e-6)
                rz_t = ep_p.tile([CHUNK, H], dt.float32, tag="rz")
                nc.vector.reciprocal(out=rz_t[:], in_=ze_t[:])
                on_t = ep_p.tile([CHUNK, D], dt.float32, tag="on")
                nc.vector.tensor_tensor(
                    out=on_t[:].rearrange("p (h d) -> p h d", d=DH),
                    in0=acc[:, 0:D].rearrange("p (h d) -> p h d", d=DH),
                    in1=rz_t[:].unsqueeze(2).to_broadcast([CHUNK, H, DH]),
                    op=mybir.AluOpType.mult)
                nc.sync.dma_start(
                    out=out_d[c * CHUNK:(c + 1) * CHUNK, :], in_=on_t[:])
    nc.compile()
    return nc


_PROGRAM_CACHE = {}
TRACE = False
LAST_RESULTS = None
LAST_GEOM = None


def kernel(**inputs):
    x = np.asarray(inputs["x"], dtype=np.float32)
    edge_attr = np.asarray(inputs["edge_attr"], dtype=np.float32)
    WQ = np.asarray(inputs["WQ"], dtype=np.float32)
    WK = np.asarray(inputs["WK"], dtype=np.float32)
    WV = np.asarray(inputs["WV"], dtype=np.float32)
    WE = np.asarray(inputs["WE"], dtype=np.float32)
    edge_index = np.asarray(inputs["edge_index"])

    per_core, shared, geom = _preprocess(
        x, edge_attr, WQ, WK, WV, WE, edge_index)
    global LAST_GEOM
    LAST_GEOM = (per_core, shared, geom)

    key = (geom["ts"], tuple(geom["calls"]))
    if key not in _PROGRAM_CACHE:
        _PROGRAM_CACHE[key] = _build_program(geom)
    nc = _PROGRAM_CACHE[key]

    in_maps = []
    for m in range(NCORES):
        im = dict(shared)
        im.update(per_core[m])
        in_maps.append({k: np.asarray(v) for k, v in im.items()})

    from concourse.bass_utils import run_bass_kernel_spmd

    res = run_bass_kernel_spmd(nc, in_maps, list(range(NCORES)), trace=TRACE)
    global LAST_RESULTS
    LAST_RESULTS = res
    out = np.empty((N, D), dtype=np.float32)
    for m in range(NCORES):
        out[m * NPC:(m + 1) * NPC] = res.results[m]["out"][:NPC]
    return out
